# revision 23
# baseline (speedup 1.0000x reference)
"""Trainium2 Bass kernel for nn_CapsLayer (CapsNet dynamic routing).

Math (per reference):
    u_hat = einsum('bid,inde->bine', x, W)    x:[64,2048,8] W:[2048,32,8,16]
    b = 0; 3 routing iters: c=softmax(b,n); s=sum_i c*u_hat; v=squash(s);
    b += sum_e u_hat*v   (iters 0,1)
    out = v [64, 32, 16]

Sharding: data-parallel over batch, 8 samples/core, W replicated.

Per-core layout (P=128 partitions, partition p = 16*b + j):
    u_hat: 32 groups [128, 4, 16, 32] bf16 (tile t: capsules i=16t..16t+15,
    free dims = (e, n)), held in a ring of tG+SPARES buffers so that rep r+1's
    einsum can run ahead while rep r's routing still reads old groups.
  - einsum: one matmul per tile: lhsT = XB_t (block-diag x, host-built),
    rhs = WR_t (re-laid W, host-built). K=(j,d), M=(j,b), N=(e,n).
  - s-reduce: lhsT [128,8] = delta[b'==b] row weights (1.0 / softmax
    normalizer R), rhs = exp-premultiplied u_hat; 4 col-group accumulation
    chains share ONE psum bank (disjoint partition rows 32c..32c+8).
  - agreement: prod = u_hat * v_bcast (vector/gpsimd split), e-reduce by
    pairwise bf16 fold-adds (2x DVE mode) or identity-matmul on PE.
  - softmax without max-subtraction: logits are bounded (|b| <~ 3), exp is
    safe in fp16.
  - squash sqrt via exp(0.5*ln(x)): keeps ACT on one table set.

The emitter takes `reps` and interleaves rep r+1's einsum phase (PE/DMA
heavy) into rep r's routing phase (DVE heavy) in program order, so the
engines pipeline across reps. Used by test.py for differential steady-state
timing; kernel() itself uses reps=1.
"""

import os
import numpy as np
import ml_dtypes

BF = np.float16

NCORES = 8
B = 8          # samples per core
I = 2048       # input capsules
J = 16         # capsules per tile
T = I // J     # 128 tiles
TG = 4         # tiles per group
D = 8          # in_dim
NN = 32        # num output capsules
E = 16         # out_dim
NE = NN * E    # 512
P = 128

USE_COLTILE = os.environ.get("K_COLTILE", "1") == "1"
POOL_EVERY = int(os.environ.get("K_POOL_EVERY", "5"))   # gpsimd gets 1 in N groups
ER1_PE = os.environ.get("K_ER1_PE", "1") == "1"         # iter-1 e-reduce on PE
SPARES = int(os.environ.get("K_SPARES", "4"))           # extra u_hat ring bufs
COPY_SPLIT = os.environ.get("K_COPY_SPLIT", "1,0,0")    # ACT,DVE,Pool copy weights
                                                        # (gpsimd cannot read PSUM;
                                                        # DVE is the bound engine, so
                                                        # ACT takes all u_hat copies)
CHUNK = int(os.environ.get("K_CH", "4"))                # tiles per DMA chunk

_CACHE = {}


# ----------------------------------------------------------------------------
# host-side input preparation
# ----------------------------------------------------------------------------

def _build_xb(xs, tT=T):
    """xs [B, I, D] f32 -> XB [128, tT*128] fp16 (p-major).
    XB[8j+d, t*128 + 16b+j] = xs[b, 16t+j, d]."""
    arr = xs.reshape(B, tT, J, D).transpose(1, 2, 0, 3)  # [t, j, b, d]
    xb = np.zeros((tT, P, P), np.float32)
    for j in range(J):
        xb[:, 8 * j:8 * j + 8, j::J] = arr[:, j].transpose(0, 2, 1)  # [t, d, b]
    return np.ascontiguousarray(xb.transpose(1, 0, 2).reshape(P, tT * P)).astype(BF)


def _build_wr(W, tT=T):
    """W [I', NN, D, E] f32 -> WR [tT, 128, 512] bf16. WR[t, 8j+d, 32e+n] = W[16t+j, n, d, e]."""
    wr = W.reshape(tT, J, NN, D, E).transpose(0, 1, 3, 4, 2)  # [t, j, d, e, n]
    wr = wr.reshape(tT, P, NE).transpose(1, 0, 2)              # [p, t, (e n)]
    return np.ascontiguousarray(wr.reshape(P, tT * NE)).astype(BF)


def _build_xw(xs, W=None, wr=None, tT=T, ch=None):
    if ch is None:
        ch = CHUNK
    """Interleave xb and wr chunk-wise into one [P, tT*(P+NE)] fp16 tensor."""
    xb = _build_xb(xs, tT)            # [P, tT*P]
    assert wr is not None
    cols = []
    for t0 in range(0, tT, ch):
        cols.append(xb[:, t0 * P:(t0 + ch) * P])
        cols.append(wr[:, t0 * NE:(t0 + ch) * NE])
    return np.ascontiguousarray(np.concatenate(cols, axis=1))


def _build_consts():
    ones8 = np.zeros((P, B), np.float32)
    ones8[np.arange(P), np.arange(P) // J] = 1.0        # delta[b'==b], p = 16b+j
    gath = np.zeros((P, B), np.float32)
    for c in range(4):
        gath[32 * c + np.arange(B), np.arange(B)] = 1.0  # sum the 4 col-group partials
    sel = np.zeros((B, P), np.float32)
    sel[np.arange(P) // J, np.arange(P)] = 1.0           # vbc row 16b+j <- v row b
    iden = np.eye(P, dtype=np.float32)
    return ones8.astype(BF), gath.astype(np.float32), sel.astype(BF), iden.astype(BF)


# ----------------------------------------------------------------------------
# kernel emission
# ----------------------------------------------------------------------------

def _emit(nc, tT=T, reps=1):
    import concourse.bass as bass
    import concourse.tile as tile
    from concourse import mybir
    from contextlib import ExitStack

    f32 = mybir.dt.float32
    bf16 = mybir.dt.float16  # 16-bit working dtype (fp16: 10-bit mantissa)
    AF = mybir.ActivationFunctionType
    AX = mybir.AxisListType
    OP = mybir.AluOpType

    tG = tT // TG
    KI = tT // 4                      # accumulation length per psum col-group
    NBUF = tG + SPARES                # u_hat ring size
    cw = [int(x) for x in COPY_SPLIT.split(",")]
    csum = sum(cw)

    xw_d = nc.dram_tensor("xw", [P, tT * (P + NE)], bf16, kind="ExternalInput").ap()
    ones8_d = nc.dram_tensor("ones8", [P, B], bf16, kind="ExternalInput").ap()
    gath_d = nc.dram_tensor("gath", [P, B], f32, kind="ExternalInput").ap()
    sel_d = nc.dram_tensor("sel", [B, P], bf16, kind="ExternalInput").ap()
    iden_d = nc.dram_tensor("iden", [P, P], bf16, kind="ExternalInput").ap()
    vout_d = nc.dram_tensor("vout", [B, NN, E], f32, kind="ExternalOutput").ap()
    DEBUG = os.environ.get("K_DEBUG", "0") == "1"
    if DEBUG:
        dbg_uh = nc.dram_tensor("dbg_uh", [P, TG, E, NN], mybir.dt.float16, kind="ExternalOutput").ap()
        dbg_sp = nc.dram_tensor("dbg_sp", [P, NE], f32, kind="ExternalOutput").ap()
        dbg_v0 = nc.dram_tensor("dbg_v0", [B, E, NN], f32, kind="ExternalOutput").ap()
        dbg_lg = nc.dram_tensor("dbg_lg", [P, 8, NN], mybir.dt.float16, kind="ExternalOutput").ap()
        dbg_vbc = nc.dram_tensor("dbg_vbc", [P, NE], mybir.dt.float16, kind="ExternalOutput").ap()

    def cap(src, ap, eoff=0):
        """Custom AP rooted at a tile/AP with extra element offset."""
        return bass.AP(tensor=src.tensor, offset=src.offset + eoff, ap=ap)

    def is_pool(g):
        # gpsimd runs products ~3.8x slower than DVE: give it ~7/32 groups
        return POOL_EVERY and (g % POOL_EVERY == POOL_EVERY - 1 or g == 2)

    def prod_eng(g):
        return nc.gpsimd if is_pool(g) else nc.vector

    with ExitStack() as ctx:
        tc = ctx.enter_context(tile.TileContext(nc))
        const = ctx.enter_context(tc.tile_pool(name="const", bufs=1))
        ones8 = const.tile([P, B], bf16, tag="ones8", name="ones8")
        nc.sync.dma_start(out=ones8, in_=ones8_d)
        gath = const.tile([P, B], f32, tag="gath", name="gath")
        nc.sync.dma_start(out=gath, in_=gath_d)
        sel = const.tile([B, P], bf16, tag="sel", name="sel")
        nc.sync.dma_start(out=sel, in_=sel_d)
        iden = const.tile([P, P], bf16, tag="iden", name="iden")
        nc.sync.dma_start(out=iden, in_=iden_d)
        nbias = const.tile([P, 1], f32, tag="nbias", name="nbias")
        nc.vector.memset(nbias, -6.0)

        pers = ctx.enter_context(tc.tile_pool(name="pers", bufs=1))
        ubuf = [pers.tile([P, TG, E, NN], bf16, tag=f"uh{i}", name=f"uh{i}")
                for i in range(NBUF)]
        logits = pers.tile([P, tT, NN], bf16, tag="logits", name="logits")
        expt = pers.tile([P, tT, NN], bf16, tag="expt", name="expt")
        zsum = pers.tile([P, tT], f32, tag="zsum", name="zsum")
        rnorm = pers.tile([P, tT], f32, tag="rnorm", name="rnorm")
        rblk = pers.tile([P, B, tT], bf16, tag="rblk", name="rblk")
        vbc = pers.tile([P, NE], bf16, tag="vbc", name="vbc")
        sp = pers.tile([P, NE], f32, tag="sp", name="sp")
        nc.vector.memset(sp, 0)

        sq = ctx.enter_context(tc.tile_pool(name="sq", bufs=1))
        agr = ctx.enter_context(tc.tile_pool(name="agr", bufs=2))
        vps = ctx.enter_context(tc.tile_pool(name="vps", bufs=1))

        # PSUM budget is 8 banks: 2 (s-accum, per-rep parity) + 2 (einsum)
        # + 2 (ssm/vbps rotating) + 2 (agreement e-reduce).
        spsum = ctx.enter_context(tc.tile_pool(name="spsum", bufs=1, space="PSUM"))
        sbank2 = [spsum.tile([P, NE], f32, tag=f"sbk{r}", name=f"sbk{r}")
                  for r in range(2)]
        smps = ctx.enter_context(tc.tile_pool(name="smps", bufs=2, space="PSUM"))
        agps = ctx.enter_context(tc.tile_pool(name="agps", bufs=2, space="PSUM"))
        ein = ctx.enter_context(tc.tile_pool(name="ein", bufs=2))
        eps = ctx.enter_context(tc.tile_pool(name="epsum", bufs=2, space="PSUM"))

        def ubi(r, g):
            return ubuf[(r * SPARES + g) % NBUF]

        # ------------------------------------------------------------------
        # phase A generator: einsum -> u_hat ring; fused iter-0 s-reduce
        # ------------------------------------------------------------------
        def phase_a_gen(r):
            sbank = sbank2[r % 2]
            CH = min(CHUNK, tT)             # tiles per DMA chunk
            CW = CH * (P + NE)
            for t0 in range(0, tT, CH):
                xwt = ein.tile([P, CW], bf16, tag="xw", name="xw")
                nc.sync.dma_start(out=xwt,
                                  in_=xw_d[:, (t0 // CH) * CW:(t0 // CH + 1) * CW])
                for tt in range(CH):
                    t = t0 + tt
                    u_slot = ubi(r, t // TG)[:, t % TG]
                    ps = eps.tile([P, NE], f32, tag="ps", name="ps")
                    nc.tensor.matmul(ps, lhsT=xwt[:, tt * P:(tt + 1) * P],
                                     rhs=xwt[:, CH * P + tt * NE:CH * P + (tt + 1) * NE],
                                     start=True, stop=True)
                    w = t % csum
                    eng = nc.scalar if w < cw[0] else (
                        nc.vector if w < cw[0] + cw[1] else nc.gpsimd)
                    psv = ps.rearrange("p (e n) -> p e n", n=NN)
                    if eng is nc.scalar:
                        nc.scalar.copy(out=u_slot, in_=psv)
                    else:
                        eng.tensor_copy(out=u_slot, in_=psv)
                    # iter-0 s-reduce (uniform c) fused into phase A
                    # block col-group mapping: chain c completes at tile
                    # 32c+31, so s_combine overlaps the s-matmul tail
                    c_, ki_ = t // KI, t % KI
                    kw0 = dict(start=(ki_ == 0), stop=(ki_ == KI - 1))
                    if USE_COLTILE:
                        kw0["tile_position"] = (0, 32 * c_)
                    nc.tensor.matmul(sbank[32 * c_:32 * c_ + B, :], lhsT=ones8,
                                     rhs=u_slot, skip_group_check=True, **kw0)
                yield

        # ------------------------------------------------------------------
        # routing helpers
        # ------------------------------------------------------------------
        def s_combine(sbank, scale):
            for c in range(4):
                nc.scalar.activation(out=sp[32 * c:32 * c + B, :],
                                     in_=sbank[32 * c:32 * c + B, :],
                                     func=AF.Copy, scale=float(scale))
            smt = smps.tile([P, NE], f32, tag="smps", name="smps")
            s_small = smt[0:B, :]
            nc.tensor.matmul(s_small, lhsT=gath, rhs=sp, start=True, stop=True)
            s_sb = sq.tile([B, NE], f32, tag="ssb", name="ssb")
            nc.scalar.copy(out=s_sb, in_=s_small)
            return s_sb

        def squash(s_small):
            """returns v_f32 [B, E, NN]; v = s * sqrt(s2)/(1+s2)."""
            s3 = s_small.rearrange("p (e n) -> p e n", n=NN)
            sqs = sq.tile([B, E, NN], f32, tag="sqs", name="sqs")
            nc.vector.tensor_mul(sqs, s3, s3)
            s2 = sq.tile([B, NN], f32, tag="s2", name="s2")
            nc.vector.tensor_reduce(s2, cap(sqs, [sqs.ap[0], [1, NN], [NN, E]]),
                                    axis=AX.X, op=OP.add)
            rt = sq.tile([B, NN], f32, tag="rt", name="rt")
            nc.scalar.activation(out=rt, in_=s2, func=AF.Ln)
            nc.scalar.activation(out=rt, in_=rt, func=AF.Exp, scale=0.5)
            den = sq.tile([B, NN], f32, tag="den", name="den")
            nc.vector.tensor_scalar_add(den, s2, 1.0)
            rec = sq.tile([B, NN], f32, tag="rec", name="rec")
            nc.vector.reciprocal(rec, den)
            scl = sq.tile([B, NN], f32, tag="scl", name="scl")
            nc.vector.tensor_mul(scl, rt, rec)
            v_f32 = vps.tile([B, E, NN], f32, tag="vf", name="vf")
            nc.vector.tensor_mul(v_f32, s3, cap(scl, [scl.ap[0], [0, E], [1, NN]]))
            return v_f32

        def bcast_v(v_f32):
            # vbc[16b+j, :] = v[b, :] via selector matmul (SEL.T @ v)
            v_bf = vps.tile([B, E, NN], bf16, tag="vb", name="vb")
            nc.vector.tensor_copy(out=v_bf, in_=v_f32)
            vps_ps = smps.tile([P, NE], f32, tag="smps", name="smps")
            nc.tensor.matmul(vps_ps, lhsT=sel,
                             rhs=cap(v_bf, [v_bf.ap[0], [1, NE]]),
                             start=True, stop=True)
            nc.scalar.copy(out=vbc, in_=vps_ps)

        def agreement_g(r, k, g, on_pe):
            eng = prod_eng(g)
            prod = agr.tile([P, TG, E, NN], bf16, tag="prod", name="prod")
            eng.tensor_mul(prod, ubi(r, g),
                           cap(vbc, [vbc.ap[0], [0, TG], [NN, E], [1, NN]]))
            lsl = logits[:, TG * g:TG * g + TG, :]
            if on_pe and not is_pool(g):
                # sum over e on PE: identity matmul with e-step-0 psum out;
                # relies on within-matmul has_written accumulation.
                aps = agps.tile([P, TG * NN], f32, tag="aps", name="aps")
                for tt in range(TG):
                    nc.tensor.matmul(
                        cap(aps, [aps.ap[0], [0, E], [1, NN]], eoff=tt * NN),
                        lhsT=iden,
                        rhs=cap(prod, [prod.ap[0], [1, NE]], eoff=tt * NE),
                        start=True, stop=True, skip_group_check=True)
                if k == 0:
                    nc.scalar.copy(out=lsl,
                                   in_=aps.rearrange("p (t n) -> p t n", n=NN))
                else:
                    a1 = agr.tile([P, TG, NN], bf16, tag="a1", name="a1")
                    nc.scalar.copy(out=a1,
                                   in_=aps.rearrange("p (t n) -> p t n", n=NN))
                    # logits accumulate on gpsimd: DVE is the bound engine
                    nc.gpsimd.tensor_add(lsl, lsl, a1)
                return
            eng.tensor_add(prod[:, :, 0:8], prod[:, :, 0:8], prod[:, :, 8:16])
            eng.tensor_add(prod[:, :, 0:4], prod[:, :, 0:4], prod[:, :, 4:8])
            eng.tensor_add(prod[:, :, 0:2], prod[:, :, 0:2], prod[:, :, 2:4])
            if k == 0:
                eng.tensor_add(lsl, prod[:, :, 0], prod[:, :, 1])
            else:
                a1 = agr.tile([P, TG, NN], bf16, tag="a1", name="a1")
                eng.tensor_add(a1, prod[:, :, 0], prod[:, :, 1])
                eng.tensor_add(lsl, lsl, a1)

        def softmax_exp(sg, SGT):
            """softmax pieces for tile range [sg*SGT, (sg+1)*SGT).
            Constant shift instead of per-row max: logits stay within ~|11|,
            so exp(x-6) cannot overflow fp16 and the shift cancels in the
            softmax normalization."""
            t0, t1 = sg * SGT, (sg + 1) * SGT
            lsl = logits[:, t0:t1, :]
            nc.scalar.activation(out=expt[:, t0:t1, :], in_=lsl, func=AF.Exp,
                                 bias=nbias)
            nc.vector.tensor_reduce(zsum[:, t0:t1], expt[:, t0:t1, :],
                                    axis=AX.X, op=OP.add)
            nc.vector.reciprocal(rnorm[:, t0:t1], zsum[:, t0:t1])
            rnh = sq.tile([P, tT], bf16, tag="rnh", name="rnh", bufs=2)
            nc.vector.tensor_copy(out=rnh[:, t0:t1], in_=rnorm[:, t0:t1])
            nc.vector.tensor_mul(
                rblk[:, :, t0:t1],
                cap(ones8, [ones8.ap[0], [1, B], [0, SGT]]),
                cap(rnh, [rnh.ap[0], [0, B], [1, SGT]], eoff=t0))

        # ------------------------------------------------------------------
        # routing generator for one rep
        # ------------------------------------------------------------------
        def routing_gen(r):
            sbank = sbank2[r % 2]
            v_f32 = squash(s_combine(sbank, 1.0 / NN))
            bcast_v(v_f32)
            yield
            for g in range(tG):
                agreement_g(r, 0, g, on_pe=False)
                if g % 4 == 3:
                    yield
            if DEBUG and r == 0:
                nc.sync.dma_start(out=dbg_uh, in_=ubi(r, 0))
                nc.sync.dma_start(out=dbg_sp, in_=sp)
                nc.sync.dma_start(out=dbg_v0, in_=v_f32)
                nc.sync.dma_start(out=dbg_vbc, in_=vbc)
                nc.sync.dma_start(out=dbg_lg, in_=logits[:, 0:8, :])

            NSG = max(1, min(4, tG))     # softmax super-groups per iteration
            SGG = tG // NSG              # groups per super-group
            SGT = SGG * TG               # tiles per super-group
            for k in (1, 2):
                for sg in range(NSG):
                    softmax_exp(sg, SGT)
                    yield
                    for g in range(sg * SGG, (sg + 1) * SGG):
                        eng = prod_eng(g)
                        prem = agr.tile([P, TG, E, NN], bf16, tag="prem", name="prem")
                        e_sl = expt[:, TG * g:TG * g + TG, :]
                        eng.tensor_mul(prem, ubi(r, g),
                                       cap(e_sl, [e_sl.ap[0], [NN, TG], [0, E], [1, NN]]))
                        for tt in range(TG):
                            t = TG * g + tt
                            c_, ki_ = t // KI, t % KI
                            kw = dict(start=(ki_ == 0), stop=(ki_ == KI - 1))
                            if USE_COLTILE:
                                kw["tile_position"] = (0, 32 * c_)
                            nc.tensor.matmul(sbank[32 * c_:32 * c_ + B, :],
                                             lhsT=rblk[:, :, t], rhs=prem[:, tt],
                                             skip_group_check=True, **kw)
                        if g % 4 == 3:
                            yield
                v_f32 = squash(s_combine(sbank, 1.0))
                if k == 1:
                    bcast_v(v_f32)
                    yield
                    for g in range(tG):
                        agreement_g(r, 1, g, on_pe=ER1_PE)
                        if g % 4 == 3:
                            yield
                else:
                    vo = vps.tile([B, NN, E], f32, tag="vo", name="vo")
                    nc.vector.tensor_copy(
                        out=vo, in_=cap(v_f32, [v_f32.ap[0], [1, NN], [NN, E]]))
                    nc.sync.dma_start(out=vout_d, in_=vo)
                    yield

        # ------------------------------------------------------------------
        # software-pipelined emission: weave rep r's phase A into rep r-1's
        # routing in program order (dependencies via the u_hat ring).
        # ------------------------------------------------------------------
        def drain(gen):
            if gen is not None:
                for _ in gen:
                    pass

        routing = None
        for r in range(reps):
            pa = phase_a_gen(r)
            if routing is None:
                drain(pa)
            else:
                # ~30 routing yields vs 16 phase-A steps: alternate 2:1
                pa_live = True
                for i, _ in enumerate(routing):
                    if pa_live and i % 2 == 1:
                        try:
                            next(pa)
                        except StopIteration:
                            pa_live = False
                if pa_live:
                    drain(pa)
            routing = routing_gen(r)
        drain(routing)

    return nc


def _get_nc(tT=T, reps=1):
    key = ("nc", tT, reps, USE_COLTILE, POOL_EVERY, ER1_PE, SPARES, COPY_SPLIT, CHUNK)
    if key not in _CACHE:
        from concourse import bacc
        nc = bacc.Bacc(trn_type="TRN2", target_bir_lowering=False, debug=False)
        _emit(nc, tT, reps=reps)
        nc.compile()
        _CACHE[key] = nc
    return _CACHE[key]


# ----------------------------------------------------------------------------
# entry point
# ----------------------------------------------------------------------------

def kernel(x, W):
    x = np.asarray(x, np.float32)
    W = np.asarray(W, np.float32)
    wr = _build_wr(W)
    ones8, gath, sel, iden = _build_consts()
    nc = _get_nc()

    in_maps = [{"xw": _build_xw(x[c * B:(c + 1) * B], wr=wr),
                "ones8": ones8, "gath": gath, "sel": sel, "iden": iden} for c in range(NCORES)]

    from concourse.bass_utils import run_bass_kernel_spmd
    res = run_bass_kernel_spmd(nc, in_maps, core_ids=list(range(NCORES)),
                               trace=False)
    out = np.concatenate([r["vout"] for r in res.results], axis=0)
    return out.astype(np.float32)


kernel.last_exec_ns = None
